# revision 1
# baseline (speedup 1.0000x reference)
"""GroupedTernaryLinear Trainium2 kernel (Bass/Tile, 8-core SPMD).

Computation (matches the jax reference):
  x:      [2, 4096, 4096] f32   -> flatten to [8192, 4096] tokens
  weight: [4096, 1024]    f32
  1. xn = rms_norm(x) over last dim (eps = f32 eps)
  2. w_bf = bf16(weight); per flat 64-chunk: scale = bf16(mean|w_bf|) (clipped),
     q = clip(round(w_bf/scale), -1, 1)  ->  wq = q*scale  (exact in bf16)
  3. out[t, g*1024+o] = sum_i xn[t, g*1024+i] * wq[g*1024+o, i]   (4 groups)

Kernel strategy:
  - Shard 8192 tokens across 8 cores (1024 each); weight replicated.
  - Quantize weight on-chip (DVE), threshold form: q = (w>t) - (w<-t) with
    t = 0.5009765625*scale (exact round-half-even bf16 equivalence).
  - PE-transpose wq -> wqT [i, o] resident in SBUF (bf16).
  - Per 128-token block: DMA x, ACT square+accum -> sumsq, PE-transpose raw
    x -> bf16 xT, then grouped matmul (lhsT = xT chunk, rhs = wqT slice),
    rms factor folded into the PSUM->SBUF output evacuation.
"""

import os
import sys

sys.path.insert(0, "/opt/trn_rl_repo")

import numpy as np

import concourse.bass as bass
import concourse.mybir as mybir
import concourse.tile as tile
from concourse import bacc
from concourse.bass_utils import run_bass_kernel_spmd
from concourse.masks import make_identity

F32 = mybir.dt.float32
BF16 = mybir.dt.bfloat16
AF = mybir.ActivationFunctionType
ALU = mybir.AluOpType

N_CORES = 8
T = 1024          # tokens per core
D = 4096          # feature dim (= 4 groups * 1024)
G = 4             # groups
GI = 1024         # group input dim
GO = 1024         # group output dim
KC = D // 128     # 32 k-chunks of 128 over the full feature dim
GK = GI // 128    # 8 k-chunks per group
TB = T // 128     # 8 token blocks per core
EPS = 1.1920929e-07          # np.finfo(np.float32).eps
THR = 0.5009765625           # bf16 round-to-nearest-even threshold for |r|>0.5

LAST_EXEC_NS = None
LAST_RESULTS = None


def _build():
    nc = bacc.Bacc("TRN2", target_bir_lowering=False, debug=False)
    x_ap = nc.dram_tensor("x", [T, D], F32, kind="ExternalInput").ap()
    w_ap = nc.dram_tensor("weight", [D, GI], F32, kind="ExternalInput").ap()
    out_ap = nc.dram_tensor("out", [T, D], F32, kind="ExternalOutput").ap()

    with tile.TileContext(nc) as tc:
        _body(tc, nc, out_ap, x_ap, w_ap)

    nc.compile()
    return nc


def _body(tc, nc, out_ap, x_ap, w_ap):
    with (
        tc.tile_pool(name="consts", bufs=1) as consts,
        tc.tile_pool(name="wqt", bufs=1) as wqt_pool,
        tc.tile_pool(name="win", bufs=2) as win_pool,
        tc.tile_pool(name="wmask", bufs=2) as wmask_pool,
        tc.tile_pool(name="xin", bufs=2) as xin_pool,
        tc.tile_pool(name="xtp", bufs=2) as xtp_pool,
        tc.tile_pool(name="stats", bufs=2) as stats_pool,
        tc.tile_pool(name="outsb", bufs=4) as out_pool,
        tc.tile_pool(name="ps_tp", bufs=2, space="PSUM") as ps_tp,
        tc.tile_pool(name="ps_wtp", bufs=2, space="PSUM") as ps_wtp,
        tc.tile_pool(name="ps_mm", bufs=2, space="PSUM") as ps_mm,
    ):
        ident_f = consts.tile([128, 128], F32, name="ident_f")
        make_identity(nc, ident_f[:])
        ident_b = consts.tile([128, 128], BF16, name="ident_b")
        make_identity(nc, ident_b[:])
        eps_t = consts.tile([128, 1], F32, name="eps_t")
        nc.vector.memset(eps_t[:], EPS)

        # Resident transposed-quantized weight: [i(128), g, k, o] bf16
        wqT = wqt_pool.tile([128, G, GK, GO], BF16, name="wqT")

        # ---------------- Phase W: quantize + transpose weight ------------
        for ow in range(D // 128):          # 32 tiles of [128 o, 1024 i]
            g, o_off = ow // 8, (ow % 8) * 128
            w_t = win_pool.tile([128, GI], F32, name="w_t")
            nc.gpsimd.dma_start(w_t[:], w_ap[ow * 128:(ow + 1) * 128, :])

            wbf = win_pool.tile([128, GI], BF16, name="wbf")
            nc.scalar.copy(wbf[:], w_t[:])              # f32 -> bf16 (RNE)

            wbf_v = wbf[:].rearrange("p (g q) -> p g q", q=64)
            red = stats_pool.tile([128, 16], F32, name="red")
            nc.vector.tensor_reduce(
                red[:], wbf_v, axis=mybir.AxisListType.X, op=ALU.add,
                apply_absolute_value=True,
            )
            s_bf = stats_pool.tile([128, 16], BF16, name="s_bf")
            nc.vector.tensor_scalar(
                s_bf[:], red[:], 1.0 / 64.0, 1e-8, ALU.mult, ALU.max,
            )
            # Materialize s_full[o, i] = s_bf[o, i//64] (bf16) and the
            # f32 thresholds +/- THR*s.
            s_full = wmask_pool.tile([128, GI], BF16, name="s_full")
            sf_v = s_full[:].rearrange("p (g q) -> p g q", q=64)
            s_b = s_bf[:].unsqueeze(2).broadcast_to((128, 16, 64))
            nc.vector.tensor_copy(sf_v, s_b)
            t_pos = wmask_pool.tile([128, GI], F32, name="t_pos")
            nc.vector.tensor_scalar_mul(t_pos[:], s_full[:], THR)
            t_neg = wmask_pool.tile([128, GI], F32, name="t_neg")
            nc.vector.tensor_scalar_mul(t_neg[:], s_full[:], -THR)

            # q = (w > t) - (w < -t); wq = q*s  (2D ops; compares on GpSimd)
            mp = wmask_pool.tile([128, GI], BF16, name="mp")
            nc.vector.tensor_tensor(mp[:], wbf[:], t_pos[:], ALU.is_gt)
            mn = wmask_pool.tile([128, GI], BF16, name="mn")
            nc.vector.tensor_tensor(mn[:], wbf[:], t_neg[:], ALU.is_lt)
            wq = wmask_pool.tile([128, GI], BF16, name="wq")
            nc.vector.tensor_sub(wq[:], mp[:], mn[:])
            nc.vector.tensor_mul(wq[:], wq[:], s_full[:])

            for k0 in range(0, GK, 4):      # 2 quads of PE transposes
                wps = ps_wtp.tile([128, 4, 128], BF16, name="wps")
                for j in range(4):
                    kk = k0 + j
                    nc.tensor.transpose(
                        wps[:, j, :], wq[:, kk * 128:(kk + 1) * 128], ident_b[:],
                    )
                nc.scalar.copy(
                    wqT[:, g, k0:k0 + 4, o_off:o_off + 128], wps[:],
                )

        # ---------------- Phase X: per 128-token block --------------------
        for tb in range(TB):
            xt = xin_pool.tile([128, D], F32, name="xt")
            nc.sync.dma_start(xt[:], x_ap[tb * 128:(tb + 1) * 128, :])

            junk = xin_pool.tile([128, D], BF16, name="junk")
            ss = stats_pool.tile([128, 1], F32, name="ss")
            nc.scalar.activation(junk[:], xt[:], AF.Square, accum_out=ss[:])
            sq = stats_pool.tile([128, 1], F32, name="sq")
            nc.scalar.activation(sq[:], ss[:], AF.Sqrt, bias=eps_t[:], scale=1.0 / D)
            fac = stats_pool.tile([128, 1], F32, name="fac")
            nc.vector.reciprocal(fac[:], sq[:])

            xT = xtp_pool.tile([128, KC, 128], BF16, name="xT")
            for c0 in range(0, KC, 4):
                xps = ps_tp.tile([128, 4, 128], F32, name="xps")
                for j in range(4):
                    cc = c0 + j
                    nc.tensor.transpose(
                        xps[:, j, :], xt[:, cc * 128:(cc + 1) * 128], ident_f[:],
                    )
                # psum f32 -> sbuf bf16 cast; alternate engines for balance
                if (c0 // 4) % 2 == 0:
                    nc.vector.tensor_copy(xT[:, c0:c0 + 4, :], xps[:])
                else:
                    nc.scalar.copy(xT[:, c0:c0 + 4, :], xps[:])

            for g in range(G):
                pm0 = ps_mm.tile([128, 512], F32, name="pm0")
                pm1 = ps_mm.tile([128, 512], F32, name="pm1")
                for k in range(GK):
                    lhsT = xT[:, g * GK + k, :]
                    nc.tensor.matmul(
                        pm0[:], lhsT, wqT[:, g, k, 0:512],
                        start=(k == 0), stop=(k == GK - 1),
                    )
                    nc.tensor.matmul(
                        pm1[:], lhsT, wqT[:, g, k, 512:1024],
                        start=(k == 0), stop=(k == GK - 1),
                    )
                # evac with rms factor folded in; split across DVE/ACT
                ob0 = out_pool.tile([128, 512], F32, name="ob0")
                nc.vector.tensor_scalar_mul(ob0[:], pm0[:], fac[:])
                nc.gpsimd.dma_start(
                    out_ap[tb * 128:(tb + 1) * 128, g * GO:g * GO + 512], ob0[:],
                )
                ob1 = out_pool.tile([128, 512], F32, name="ob1")
                nc.vector.tensor_scalar_mul(ob1[:], pm1[:], fac[:])
                nc.gpsimd.dma_start(
                    out_ap[tb * 128:(tb + 1) * 128, g * GO + 512:(g + 1) * GO],
                    ob1[:],
                )


_NC_CACHE = None


def _ensure_ntff_hook():
    """Install the antenv.axon_hooks shim + ctypes NTFF hook if missing.

    Some images lack ``antenv.axon_hooks``; bass_utils imports it
    unconditionally when trace=True under axon. Build the module in-memory
    and register the boot shim's ctypes-based hook.
    """
    import types

    try:
        from antenv.axon_hooks import get_axon_ntff_profile_hook  # noqa: F401
        return
    except ImportError:
        pass
    import antenv

    mod = types.ModuleType("antenv.axon_hooks")
    mod._hook = None
    mod.set_axon_ntff_profile_hook = lambda h: setattr(mod, "_hook", h)
    mod.get_axon_ntff_profile_hook = lambda: mod._hook
    sys.modules["antenv.axon_hooks"] = mod
    antenv.axon_hooks = mod
    try:
        if "/root/.axon_site" not in sys.path:
            sys.path.insert(0, "/root/.axon_site")
        from trn_agent_boot.trn_boot import _ntff_profile_via_ctypes

        mod.set_axon_ntff_profile_hook(
            _ntff_profile_via_ctypes("/opt/axon/libaxon_pjrt.so")
        )
    except Exception:
        pass


def kernel(x: np.ndarray, weight: np.ndarray) -> np.ndarray:
    global LAST_EXEC_NS, LAST_RESULTS, _NC_CACHE
    x = np.ascontiguousarray(np.asarray(x, dtype=np.float32))
    weight = np.ascontiguousarray(np.asarray(weight, dtype=np.float32))
    lead = x.shape[:-1]
    xf = x.reshape(-1, D)
    assert xf.shape[0] == N_CORES * T, xf.shape

    if _NC_CACHE is None:
        _NC_CACHE = _build()
    nc = _NC_CACHE

    in_maps = [
        {"x": xf[i * T:(i + 1) * T], "weight": weight} for i in range(N_CORES)
    ]
    trace = bool(int(os.environ.get("CCK_TRACE", "0")))
    kw = {}
    if trace:
        _ensure_ntff_hook()
        tdir = os.environ.get("CCK_TRACE_DIR")
        if tdir:
            os.makedirs(tdir, exist_ok=True)
            kw["tmpdir"] = tdir
    res = run_bass_kernel_spmd(nc, in_maps, list(range(N_CORES)), trace=trace, **kw)
    LAST_EXEC_NS = res.exec_time_ns
    LAST_RESULTS = res
    out = np.concatenate([res.results[i]["out"] for i in range(N_CORES)], axis=0)
    return out.reshape(*lead, D).astype(np.float32, copy=False)


if __name__ == "__main__":
    rng = np.random.default_rng(0)
    x = rng.standard_normal((2, 4096, 4096), dtype=np.float32)
    w = (rng.standard_normal((4096, 1024), dtype=np.float32) * 0.02).astype(np.float32)
    o = kernel(x, w)
    print(o.shape, o.dtype, LAST_EXEC_NS)



# revision 4
# speedup vs baseline: 1.7695x; 1.7695x over previous
"""GroupedTernaryLinear Trainium2 kernel (Bass/Tile, 8-core SPMD).

Computation (matches the jax reference):
  x:      [2, 4096, 4096] f32   -> flatten to [8192, 4096] tokens
  weight: [4096, 1024]    f32
  1. xn = rms_norm(x) over last dim (eps = f32 eps)
  2. w_bf = bf16(weight); per flat 64-chunk: scale = bf16(mean|w_bf|) (clipped),
     q = clip(round(w_bf/scale), -1, 1)  ->  wq = q*scale  (exact in bf16)
  3. out[t, g*1024+o] = sum_i xn[t, g*1024+i] * wq[g*1024+o, i]   (4 groups)

Sharding: 2 token-halves x 4 groups = 8 cores. Core c = 4*i + j gets
tokens [4096*i, 4096*(i+1)) and group j (x feature slice j*1024..,
weight rows j*1024..). Each core quantizes only ITS group's weight
(1/4 of the work vs replicating). The rms-norm sum of squares needs all
4096 features per token, so the 4 group-shards of a token half exchange
per-token partial sums with a tiny (16 KB) AllReduce; the norm factor is
folded into the PSUM evacuation of the matmul outputs.

Engine layout (per 128-token block / 128-row weight tile):
  PE:     wq transposes, x transposes (bf16), 512-col matmuls
  ACT:    w bf16 cast, wqT psum->sbuf, x square+accum, xT psum->sbuf, sqrt
  DVE:    quant reduce/scale/thresholds/sub/mul, x bf16 cast, psum->raw evac,
          reciprocal
  GPSIMD: quant compares, AllReduce, final raw*fac -> out, out DMA
"""

import os
import sys

sys.path.insert(0, "/opt/trn_rl_repo")

import numpy as np

import concourse.bass as bass
import concourse.mybir as mybir
import concourse.tile as tile
from concourse import bacc
from concourse.bass_utils import run_bass_kernel_spmd
from concourse.masks import make_identity

F32 = mybir.dt.float32
BF16 = mybir.dt.bfloat16
AF = mybir.ActivationFunctionType
ALU = mybir.AluOpType

N_CORES = 8
TOK = 4096        # tokens per core
DIN = 1024        # per-core input features (one group)
DOUT = 1024       # per-core outputs (one group)
DFULL = 4096      # full feature dim (norm denominator)
TB = TOK // 128   # 32 token blocks
NT = DOUT // 128  # 8 weight tiles of [128 o, 1024 i]
GK = DIN // 128   # 8 k-chunks of 128
EPS = 1.1920929e-07          # np.finfo(np.float32).eps
THR = 0.5009765625           # bf16 round-to-nearest-even threshold for |r|>0.5

LAST_EXEC_NS = None
LAST_RESULTS = None


def _build():
    nc = bacc.Bacc("TRN2", target_bir_lowering=False, debug=False, num_devices=8)
    x_ap = nc.dram_tensor("x", [TOK, DIN], F32, kind="ExternalInput").ap()
    w_ap = nc.dram_tensor("weight", [DOUT, DIN], F32, kind="ExternalInput").ap()
    out_ap = nc.dram_tensor("out", [TOK, DOUT], F32, kind="ExternalOutput").ap()

    with tile.TileContext(nc) as tc:
        _body(tc, nc, out_ap, x_ap, w_ap)

    nc.compile()
    return nc


def _body(tc, nc, out_ap, x_ap, w_ap):
    with (
        tc.tile_pool(name="consts", bufs=1) as consts,
        tc.tile_pool(name="wqt", bufs=1) as wqt_pool,
        tc.tile_pool(name="xta", bufs=1) as xta_pool,
        tc.tile_pool(name="win", bufs=2) as win_pool,
        tc.tile_pool(name="wtmp", bufs=2) as wtmp_pool,
        tc.tile_pool(name="wst", bufs=2) as wst_pool,
        tc.tile_pool(name="xin", bufs=4) as xin_pool,
        tc.tile_pool(name="xbfp", bufs=3) as xbf_pool,
        tc.tile_pool(name="stats", bufs=1) as stats_pool,
        tc.tile_pool(name="rawp", bufs=18) as raw_pool,
        tc.tile_pool(name="obp", bufs=4) as ob_pool,
        tc.tile_pool(name="dram", bufs=1, space="DRAM") as dram_pool,
        tc.tile_pool(name="ps_mm", bufs=2, space="PSUM") as ps_mm,
        tc.tile_pool(name="ps_xtp", bufs=2, space="PSUM") as ps_xtp,
        tc.tile_pool(name="ps_wtp", bufs=2, space="PSUM") as ps_wtp,
    ):
        ident_b = consts.tile([128, 128], BF16, name="ident_b")
        make_identity(nc, ident_b[:])
        eps_t = consts.tile([128, 1], F32, name="eps_t")
        nc.vector.memset(eps_t[:], EPS)

        # Resident transposed-quantized weight: [i(128), k, o] bf16 (16KB/part)
        wqT = wqt_pool.tile([128, GK, DOUT], BF16, name="wqT")
        # All x blocks transposed: [i(128), b, k, t] bf16 (64KB/part)
        xTa = xta_pool.tile([128, TB, GK, 128], BF16, name="xTa")

        ss_all = stats_pool.tile([128, TB], F32, name="ss_all")
        ss_sum = stats_pool.tile([128, TB], F32, name="ss_sum")
        sq_all = stats_pool.tile([128, TB], F32, name="sq_all")
        fac_all = stats_pool.tile([128, TB], F32, name="fac_all")
        junk = stats_pool.tile([128, DIN], BF16, name="junk")

        cc_in = dram_pool.tile([128, TB], F32, name="cc_in")
        cc_out = dram_pool.tile([128, TB], F32, name="cc_out")

        raws = []

        def emit_wtile(t):
            w_t = win_pool.tile([128, DIN], F32, name="w_t")
            nc.sync.dma_start(w_t[:], w_ap[t * 128:(t + 1) * 128, :])

            wbf = wtmp_pool.tile([128, DIN], BF16, name="wbf")
            nc.scalar.copy(wbf[:], w_t[:])              # f32 -> bf16 (RNE)
            wbf_v = wbf[:].rearrange("p (c q) -> p c q", q=64)

            red = wst_pool.tile([128, 16], F32, name="red")
            nc.vector.tensor_reduce(
                red[:], wbf_v, axis=mybir.AxisListType.X, op=ALU.add,
                apply_absolute_value=True,
            )
            s_bf = wst_pool.tile([128, 16], BF16, name="s_bf")
            nc.vector.tensor_scalar(
                s_bf[:], red[:], 1.0 / 64.0, 1e-8, ALU.mult, ALU.max,
            )
            thr_p = wst_pool.tile([128, 16], F32, name="thr_p")
            nc.vector.tensor_scalar_mul(thr_p[:], s_bf[:], THR)
            thr_n = wst_pool.tile([128, 16], F32, name="thr_n")
            nc.vector.tensor_scalar_mul(thr_n[:], s_bf[:], -THR)

            # q = (w > t) - (w < -t); wq = q*s  (broadcast views, no
            # materialized full-width scale/threshold tensors)
            tp_b = thr_p[:].unsqueeze(2).broadcast_to((128, 16, 64))
            tn_b = thr_n[:].unsqueeze(2).broadcast_to((128, 16, 64))
            s_b = s_bf[:].unsqueeze(2).broadcast_to((128, 16, 64))
            mp = wtmp_pool.tile([128, DIN], BF16, name="mp")
            mp_v = mp[:].rearrange("p (c q) -> p c q", q=64)
            nc.vector.tensor_tensor(mp_v, wbf_v, tp_b, ALU.is_gt)
            mn = wtmp_pool.tile([128, DIN], BF16, name="mn")
            mn_v = mn[:].rearrange("p (c q) -> p c q", q=64)
            nc.vector.tensor_tensor(mn_v, wbf_v, tn_b, ALU.is_lt)
            nc.vector.tensor_sub(mp[:], mp[:], mn[:])
            wqv = wtmp_pool.tile([128, DIN], BF16, name="wqv")
            wqv_v = wqv[:].rearrange("p (c q) -> p c q", q=64)
            nc.vector.tensor_tensor(wqv_v, mp_v, s_b, ALU.mult)

            wps = ps_wtp.tile([128, GK, 128], BF16, name="wps")
            for k in range(GK):
                nc.tensor.transpose(
                    wps[:, k, :], wqv[:, k * 128:(k + 1) * 128], ident_b[:],
                )
            nc.scalar.copy(wqT[:, :, t * 128:(t + 1) * 128], wps[:])

        def emit_xblock(b):
            x_t = xin_pool.tile([128, DIN], F32, name="x_t")
            nc.sync.dma_start(x_t[:], x_ap[b * 128:(b + 1) * 128, :])
            nc.scalar.activation(
                junk[:], x_t[:], AF.Square, accum_out=ss_all[:, b:b + 1],
            )
            xb = xbf_pool.tile([128, DIN], BF16, name="xb")
            nc.vector.tensor_copy(xb[:], x_t[:])
            xps = ps_xtp.tile([128, GK, 128], BF16, name="xps")
            for k in range(GK):
                nc.tensor.transpose(
                    xps[:, k, :], xb[:, k * 128:(k + 1) * 128], ident_b[:],
                )
            nc.scalar.copy(xTa[:, b], xps[:])

        def emit_mm(b):
            pm0 = ps_mm.tile([128, 512], F32, name="pm0")
            pm1 = ps_mm.tile([128, 512], F32, name="pm1")
            for k in range(GK):
                lhsT = xTa[:, b, k, :]
                nc.tensor.matmul(
                    pm0[:], lhsT, wqT[:, k, 0:512],
                    start=(k == 0), stop=(k == GK - 1),
                )
                nc.tensor.matmul(
                    pm1[:], lhsT, wqT[:, k, 512:1024],
                    start=(k == 0), stop=(k == GK - 1),
                )
            raw = raw_pool.tile([128, DOUT], BF16, name="raw")
            nc.vector.tensor_copy(raw[:, 0:512], pm0[:])
            nc.vector.tensor_copy(raw[:, 512:1024], pm1[:])
            raws.append(raw)

        def emit_finale(b):
            ob = ob_pool.tile([128, DOUT], F32, name="ob")
            nc.scalar.activation(
                ob[:], raws[b][:], AF.Copy, scale=fac_all[:, b:b + 1],
            )
            nc.gpsimd.dma_start(out_ap[b * 128:(b + 1) * 128, :], ob[:])

        # ---- emission order ------------------------------------------------
        # Weight tiles interleaved with the first 16 x blocks: PE does the
        # weight + x transposes while DVE/GPSIMD quantize; matmuls only enter
        # the PE queue after every weight transpose, so the PE FIFO can't
        # deadlock on wqT and matmul b starts as soon as wqT completes.
        for t in range(NT):
            emit_wtile(t)
            emit_xblock(2 * t)
            emit_xblock(2 * t + 1)
        for i, b in enumerate(range(16, TB)):
            emit_xblock(b)
            emit_mm(i)
        # Cross-core sum of squares (4 group-shards of this token half).
        nc.sync.dma_start(cc_in[:], ss_all[:])
        nc.gpsimd.collective_compute(
            "AllReduce",
            ALU.add,
            replica_groups=[[0, 1, 2, 3], [4, 5, 6, 7]],
            ins=[cc_in.opt()],
            outs=[cc_out.opt()],
        )
        nc.sync.dma_start(ss_sum[:], cc_out[:])
        nc.scalar.activation(
            sq_all[:], ss_sum[:], AF.Sqrt, bias=eps_t[:], scale=1.0 / DFULL,
        )
        nc.vector.reciprocal(fac_all[:], sq_all[:])
        for b in range(16, TB):
            emit_mm(b)
        for b in range(TB):
            emit_finale(b)


_NC_CACHE = None


def _ensure_ntff_hook():
    """Install the antenv.axon_hooks shim + ctypes NTFF hook if missing."""
    import types

    try:
        from antenv.axon_hooks import get_axon_ntff_profile_hook  # noqa: F401
        return
    except ImportError:
        pass
    import antenv

    mod = types.ModuleType("antenv.axon_hooks")
    mod._hook = None
    mod.set_axon_ntff_profile_hook = lambda h: setattr(mod, "_hook", h)
    mod.get_axon_ntff_profile_hook = lambda: mod._hook
    sys.modules["antenv.axon_hooks"] = mod
    antenv.axon_hooks = mod
    try:
        if "/root/.axon_site" not in sys.path:
            sys.path.insert(0, "/root/.axon_site")
        from trn_agent_boot.trn_boot import _ntff_profile_via_ctypes

        mod.set_axon_ntff_profile_hook(
            _ntff_profile_via_ctypes("/opt/axon/libaxon_pjrt.so")
        )
    except Exception:
        pass


def kernel(x: np.ndarray, weight: np.ndarray) -> np.ndarray:
    global LAST_EXEC_NS, LAST_RESULTS, _NC_CACHE
    x = np.ascontiguousarray(np.asarray(x, dtype=np.float32))
    weight = np.ascontiguousarray(np.asarray(weight, dtype=np.float32))
    lead = x.shape[:-1]
    xf = x.reshape(-1, DFULL)
    assert xf.shape[0] == 2 * TOK, xf.shape

    if _NC_CACHE is None:
        _NC_CACHE = _build()
    nc = _NC_CACHE

    in_maps = []
    for i in range(2):
        for j in range(4):
            in_maps.append({
                "x": np.ascontiguousarray(
                    xf[i * TOK:(i + 1) * TOK, j * DIN:(j + 1) * DIN]
                ),
                "weight": np.ascontiguousarray(
                    weight[j * DOUT:(j + 1) * DOUT, :]
                ),
            })
    trace = bool(int(os.environ.get("CCK_TRACE", "0")))
    kw = {}
    if trace:
        _ensure_ntff_hook()
        tdir = os.environ.get("CCK_TRACE_DIR")
        if tdir:
            os.makedirs(tdir, exist_ok=True)
            kw["tmpdir"] = tdir
    res = run_bass_kernel_spmd(nc, in_maps, list(range(N_CORES)), trace=trace, **kw)
    LAST_EXEC_NS = res.exec_time_ns
    LAST_RESULTS = res
    out = np.empty((2 * TOK, DFULL), dtype=np.float32)
    for i in range(2):
        for j in range(4):
            out[i * TOK:(i + 1) * TOK, j * DOUT:(j + 1) * DOUT] = (
                res.results[i * 4 + j]["out"]
            )
    return out.reshape(*lead, DFULL)


if __name__ == "__main__":
    rng = np.random.default_rng(0)
    x = rng.standard_normal((2, 4096, 4096), dtype=np.float32)
    w = (rng.standard_normal((4096, 1024), dtype=np.float32) * 0.02).astype(np.float32)
    o = kernel(x, w)
    print(o.shape, o.dtype, LAST_EXEC_NS)
